# revision 8
# baseline (speedup 1.0000x reference)
"""Multi-head dilated sliding-window attention (window=129, dil=1) on 8 TRN2 cores.

Sharding: sequence-parallel. Each core computes 256 query rows (N=2048 / 8),
with a 64-row K/V halo on each side (zero-padded at the sequence edges).
Weights are replicated (streamed from HBM once per core, bf16).

Band-softmax identity used (reference softmaxes the FULL row with zeros
outside the band):
    out_i = (sum_band (e^{s_ij} - 1) V_j + sum_all V_j) / (sum_band (e^{s_ij} - 1) + N)
computed per head with V_raw = x@Wv (no bias; bv is folded in after the
attention average), bk applied only to real (non-padding) K rows via an
indicator-row matmul, and the global sum_all V_j = (sum_n x_n) @ Wv computed
on-device from the (tiny) host-reduced x column-sum.

Compute dtype: bf16 operands into the PE (fp32 runs at quarter rate on TRN2 —
two half-speed passes), fp32 PSUM accumulation and fp32 softmax arithmetic.
"""

import numpy as np
import ml_dtypes
from contextlib import ExitStack

import concourse.bass as bass
import concourse.tile as tile
from concourse import bacc, mybir
from concourse.bass_utils import run_bass_kernel_spmd
from concourse.masks import make_identity

F32 = mybir.dt.float32
BF16 = mybir.dt.bfloat16
NPBF16 = ml_dtypes.bfloat16
N, E, H, D = 2048, 1024, 16, 64
R = N // 8          # 256 query rows per core
HALO = R + 128      # 384 K/V rows per core
NQB = R // 128      # query blocks per core


def build_graph():
    nc = bacc.Bacc("TRN2", target_bir_lowering=False, debug=False, num_devices=8)

    xh_d = nc.declare_dram_parameter("xh", [HALO, E], BF16, isOutput=False)
    xvalid_d = nc.declare_dram_parameter("xvalid", [1, HALO], BF16, isOutput=False)
    wq_d = nc.declare_dram_parameter("Wq", [E, H * D], BF16, isOutput=False)
    wk_d = nc.declare_dram_parameter("Wk", [E, H * D], BF16, isOutput=False)
    wv_d = nc.declare_dram_parameter("Wv", [E, H * D], BF16, isOutput=False)
    wo_d = nc.declare_dram_parameter("Wo", [H * D, E], BF16, isOutput=False)
    bq_d = nc.declare_dram_parameter("bq_r", [128, 8], F32, isOutput=False)
    bk_d = nc.declare_dram_parameter("bk_row", [1, H * D], BF16, isOutput=False)
    bv_d = nc.declare_dram_parameter("bv_r", [128, 8], F32, isOutput=False)
    bo_d = nc.declare_dram_parameter("bo_row", [1, E], BF16, isOutput=False)
    xsum_d = nc.declare_dram_parameter("xsum_r", [128, 8], BF16, isOutput=False)
    m0_d = nc.declare_dram_parameter("mask0", [128, 128], BF16, isOutput=False)
    m1_d = nc.declare_dram_parameter("mask1", [128, 128], BF16, isOutput=False)
    out_d = nc.declare_dram_parameter("out", [R, E], F32, isOutput=True)

    with tile.TileContext(nc) as tc, ExitStack() as ctx:
        const = ctx.enter_context(tc.tile_pool(name="const", bufs=1))
        pers = ctx.enter_context(tc.tile_pool(name="pers", bufs=1))
        wpool = ctx.enter_context(tc.tile_pool(name="wpool", bufs=4))
        epool = ctx.enter_context(tc.tile_pool(name="epool", bufs=6))
        ppool = ctx.enter_context(tc.tile_pool(name="ppool", bufs=6))
        zpool = ctx.enter_context(tc.tile_pool(name="zpool", bufs=4))
        obpool = ctx.enter_context(tc.tile_pool(name="obpool", bufs=2))
        psum = ctx.enter_context(tc.tile_pool(name="psum", bufs=8, space="PSUM"))

        def ps(shape, dt=F32):
            return psum.tile(shape, dt, tag="ps", name="pst")

        # ---- constants & small inputs -------------------------------------
        identity = const.tile([128, 128], BF16, tag="identity")
        make_identity(nc, identity[:])
        m0 = const.tile([128, 128], BF16, tag="m0")
        nc.sync.dma_start(m0[:], m0_d[:, :])
        m1 = const.tile([128, 128], BF16, tag="m1")
        nc.sync.dma_start(m1[:], m1_d[:, :])
        bq_sb = const.tile([128, 8], F32, tag="bq")
        nc.sync.dma_start(bq_sb[:], bq_d[:, :])
        bv_sb = const.tile([128, 8], F32, tag="bv")
        nc.sync.dma_start(bv_sb[:], bv_d[:, :])
        xsum_sb = const.tile([128, 8], BF16, tag="xsum")
        nc.sync.dma_start(xsum_sb[:], xsum_d[:, :])
        bk_sb = const.tile([1, H * D], BF16, tag="bk")
        nc.sync.dma_start(bk_sb[:], bk_d[:, :])
        bo_sb = const.tile([1, E], BF16, tag="bo")
        nc.sync.dma_start(bo_sb[:], bo_d[:, :])
        valid_sb = const.tile([1, HALO], BF16, tag="valid")
        nc.sync.dma_start(valid_sb[:], xvalid_d[:, :])
        ones_sb = const.tile([1, 128], BF16, tag="ones")
        nc.vector.memset(ones_sb[:], 1.0)
        biascat = const.tile([1, H, D + 1], BF16, tag="biascat")

        # ---- persistent activations ---------------------------------------
        xT = pers.tile([128, 8, HALO], BF16, tag="xT")       # [e_p, e_t, seq]
        QT = pers.tile([128, 8, R], BF16, tag="QT")          # [d_p, d_t, q]
        KT = pers.tile([128, 8, HALO], BF16, tag="KT")       # [d_p, d_t, seq]
        Vaug = pers.tile([128, 3, H, D + 1], BF16, tag="Vaug")
        Asc = pers.tile([128, NQB, H * D], BF16, tag="Asc")  # [q_p, qblk, dims]
        AT = pers.tile([128, 8, R], BF16, tag="AT")          # [d_p, d_t, q]

        # ---- load x and transpose to xT (PE transpose path) ---------------
        USE_DMA_TRANSPOSE = False
        if USE_DMA_TRANSPOSE:
            for et in range(8):
                nc.sync.dma_start(xT[:, et, :], xh_d[:, et * 128:(et + 1) * 128],
                                  transpose=True)
        else:
            xtiles = []
            for st in range(3):
                xt = wpool.tile([128, E], BF16, tag="xload", name="xt")
                nc.sync.dma_start(xt[:], xh_d[st * 128:(st + 1) * 128, :])
                xtiles.append(xt)
            for st in range(3):
                for et in range(8):
                    tp = ps([128, 128], BF16)
                    nc.tensor.transpose(tp[:], xtiles[st][:, et * 128:(et + 1) * 128],
                                        identity[:])
                    nc.scalar.copy(xT[:, et, st * 128:(st + 1) * 128], tp[:])

        # ---- Q^T projection (dims on partitions), bias bq on copyback -----
        qps = [ps([128, R]) for _ in range(8)]
        for et in range(8):
            wt = wpool.tile([128, H * D], BF16, tag="w")
            nc.sync.dma_start(wt[:], wq_d[et * 128:(et + 1) * 128, :])
            for db in range(8):
                nc.tensor.matmul(qps[db][:], wt[:, db * 128:(db + 1) * 128],
                                 xT[:, et, 64:64 + R],
                                 start=(et == 0), stop=(et == 7))
        for db in range(8):
            nc.scalar.add(QT[:, db, :], qps[db][:], bq_sb[:, db:db + 1])

        # ---- K^T projection; bk added only on real rows via indicator mm --
        kps = [ps([128, HALO]) for _ in range(8)]
        for et in range(8):
            wt = wpool.tile([128, H * D], BF16, tag="w")
            nc.sync.dma_start(wt[:], wk_d[et * 128:(et + 1) * 128, :])
            for db in range(8):
                nc.tensor.matmul(kps[db][:], wt[:, db * 128:(db + 1) * 128],
                                 xT[:, et, :], start=(et == 0), stop=False)
        for db in range(8):
            nc.tensor.matmul(kps[db][:], bk_sb[0:1, db * 128:(db + 1) * 128],
                             valid_sb[0:1, :], start=False, stop=True)
        for db in range(8):
            nc.scalar.copy(KT[:, db, :], kps[db][:])

        # ---- V (natural layout, raw: bv folded into output stage) ---------
        # also S_V = xsum @ Wv accumulated in a [1, 1024] psum pair
        vps = [ps([128, 512]) for _ in range(6)]
        svps = [ps([1, 512]) for _ in range(2)]
        for et in range(8):
            wt = wpool.tile([128, H * D], BF16, tag="w")
            nc.sync.dma_start(wt[:], wv_d[et * 128:(et + 1) * 128, :])
            for st in range(3):
                for hf in range(2):
                    nc.tensor.matmul(vps[st * 2 + hf][:],
                                     xT[:, et, st * 128:(st + 1) * 128],
                                     wt[:, hf * 512:(hf + 1) * 512],
                                     start=(et == 0), stop=(et == 7))
            for hf in range(2):
                nc.tensor.matmul(svps[hf][:], xsum_sb[:, et:et + 1],
                                 wt[:, hf * 512:(hf + 1) * 512],
                                 start=(et == 0), stop=(et == 7))
        for st in range(3):
            for hf in range(2):
                src = vps[st * 2 + hf][:].rearrange("p (h d) -> p h d", d=D)
                nc.scalar.copy(Vaug[:, st, hf * 8:(hf + 1) * 8, 0:D], src)
        nc.vector.memset(Vaug[:, :, :, D:D + 1], 1.0)
        for hf in range(2):
            src = svps[hf][:].rearrange("p (h d) -> p h d", d=D)
            nc.scalar.copy(biascat[:, hf * 8:(hf + 1) * 8, 0:D], src)
        nc.vector.memset(biascat[:, :, D:D + 1], 2048.0)

        # ---- banded attention (software-pipelined, lag PV behind S) -------
        # Per head: S^T = K^T.T @ Q^T (2 matmuls) -> exp on ACT -> (-1) on
        # GpSimd -> band-mask multiply on DVE -> PV matmuls. Emitting the PV
        # matmuls LAG heads behind the S matmuls keeps the PE streaming
        # instead of stalling on the ACT/DVE chain between S and PV.
        LAG = 2
        for qblk in range(NQB):
            ptiles = {}
            pvts = {}

            def smm_stage(h):
                db, r = h // 2, (h % 2) * 64
                ptiles[h] = []
                for cblk in range(2):
                    sp = ps([128, 128])
                    nc.tensor.matmul(
                        sp[:],
                        KT[r:r + 64, db, (qblk + cblk) * 128:(qblk + cblk + 1) * 128],
                        QT[r:r + 64, db, qblk * 128:(qblk + 1) * 128],
                        start=True, stop=True)
                    et_ = epool.tile([128, 128], BF16, tag="e", name="et_")
                    nc.scalar.activation(et_[:], sp[:],
                                         mybir.ActivationFunctionType.Exp)
                    nc.gpsimd.tensor_scalar_add(et_[:], et_[:], -1.0)
                    pt = ppool.tile([128, 128], BF16, tag="p", name="pt")
                    nc.vector.tensor_mul(pt[:], et_[:],
                                         (m0 if cblk == 0 else m1)[:])
                    ptiles[h].append(pt)

            def pv_stage(h):
                pair = h // 2
                if h % 2 == 0:
                    pvts[pair] = ps([128, 2 * (D + 1)])
                pv = pvts[pair]
                off = (h % 2) * (D + 1)
                for cblk in range(2):
                    # one accumulation group per PSUM bank (two heads share
                    # a bank): only the bank's first matmul starts it
                    nc.tensor.matmul(pv[:, off:off + D + 1],
                                     ptiles[h][cblk][:],
                                     Vaug[:, qblk + cblk, h, :],
                                     start=(h % 2 == 0 and cblk == 0),
                                     stop=False)
                if h % 2 == 1:
                    # [S_V | 2048] added to every query row (rank-1 ones mm)
                    for hh in (h - 1, h):
                        offh = (hh % 2) * (D + 1)
                        nc.tensor.matmul(pv[:, offh:offh + D + 1],
                                         ones_sb[0:1, :], biascat[0:1, hh, :],
                                         start=False, stop=(hh % 2 == 1))
                    for hh in (h - 1, h):
                        offh = (hh % 2) * (D + 1)
                        zinv = zpool.tile([128, 1], F32, tag="z", name="zinv")
                        nc.vector.reciprocal(zinv[:], pv[:, offh + D:offh + D + 1])
                        nc.vector.tensor_scalar_mul(
                            Asc[:, qblk, hh * D:(hh + 1) * D],
                            pv[:, offh:offh + D], zinv[:])
                    del ptiles[h - 1], ptiles[h], pvts[pair]

            for step in range(H + LAG):
                if step < H:
                    smm_stage(step)
                if step >= LAG:
                    pv_stage(step - LAG)

        # ---- transpose A (and add bv) for the output projection -----------
        for qblk in range(NQB):
            for at in range(8):
                tp = ps([128, 128], BF16)
                nc.tensor.transpose(tp[:], Asc[:, qblk, at * 128:(at + 1) * 128],
                                    identity[:])
                nc.scalar.add(AT[:, at, qblk * 128:(qblk + 1) * 128], tp[:],
                              bv_sb[:, at:at + 1])

        # ---- output projection: O = (A + bv) @ Wo + bo --------------------
        ops = [ps([128, 512]) for _ in range(2 * NQB)]
        for qblk in range(NQB):
            for hf in range(2):
                nc.tensor.matmul(ops[qblk * 2 + hf][:], ones_sb[0:1, :],
                                 bo_sb[0:1, hf * 512:(hf + 1) * 512],
                                 start=True, stop=False)
        for at in range(8):
            wt = wpool.tile([128, E], BF16, tag="w")
            nc.sync.dma_start(wt[:], wo_d[at * 128:(at + 1) * 128, :])
            for qblk in range(NQB):
                for hf in range(2):
                    nc.tensor.matmul(ops[qblk * 2 + hf][:],
                                     AT[:, at, qblk * 128:(qblk + 1) * 128],
                                     wt[:, hf * 512:(hf + 1) * 512],
                                     start=False, stop=(at == 7))
        for qblk in range(NQB):
            ob = obpool.tile([128, E], F32, tag="ob")
            for hf in range(2):
                nc.scalar.copy(ob[:, hf * 512:(hf + 1) * 512],
                               ops[qblk * 2 + hf][:])
            nc.sync.dma_start(out_d[qblk * 128:(qblk + 1) * 128, :], ob[:])

    nc.compile()
    return nc


_NC = None


def get_nc():
    global _NC
    if _NC is None:
        _NC = build_graph()
    return _NC


def make_in_maps(x, Wq, bq, Wk, bk, Wv, bv, Wo, bo):
    f = lambda a: np.ascontiguousarray(np.asarray(a, dtype=np.float32))
    bf = lambda a: np.ascontiguousarray(
        np.asarray(a, dtype=np.float32).astype(NPBF16))
    x2 = f(x).reshape(N, E)
    ci = np.arange(128, dtype=np.float32)[:, None]
    qi = np.arange(128, dtype=np.float32)[None, :]
    common = {
        "Wq": bf(Wq), "Wk": bf(Wk), "Wv": bf(Wv), "Wo": bf(Wo),
        "bq_r": f(bq).reshape(8, 128).T.copy(),
        "bk_row": bf(bk).reshape(1, H * D),
        "bv_r": f(bv).reshape(8, 128).T.copy(),
        "bo_row": bf(bo).reshape(1, E),
        "xsum_r": bf(x2.sum(0, dtype=np.float32)).reshape(8, 128).T.copy(),
        "mask0": (ci >= qi).astype(NPBF16),
        "mask1": (ci <= qi).astype(NPBF16),
    }
    in_maps = []
    for c in range(8):
        r0 = c * R
        xh = np.zeros((HALO, E), NPBF16)
        valid = np.zeros((1, HALO), NPBF16)
        lo, hi = r0 - 64, r0 + R + 64
        slo, shi = max(lo, 0), min(hi, N)
        xh[slo - lo: shi - lo] = x2[slo:shi].astype(NPBF16)
        valid[0, slo - lo: shi - lo] = 1.0
        in_maps.append({**common, "xh": xh, "xvalid": valid})
    return in_maps


def kernel(x, Wq, bq, Wk, bk, Wv, bv, Wo, bo, _trace=False, _trace_kwargs=None):
    nc = get_nc()
    in_maps = make_in_maps(x, Wq, bq, Wk, bk, Wv, bv, Wo, bo)
    res = run_bass_kernel_spmd(nc, in_maps, list(range(8)), trace=_trace,
                               **(_trace_kwargs or {}))
    out = np.concatenate([res.results[c]["out"] for c in range(8)], axis=0)
    kernel.last_result = res
    return out[None].astype(np.float32)


# revision 9
# speedup vs baseline: 1.5619x; 1.5619x over previous
"""Multi-head dilated sliding-window attention (window=129, dil=1) on 8 TRN2 cores.

Sharding: sequence-parallel. Each core computes 256 query rows (N=2048 / 8),
with a 64-row K/V halo on each side (zero-padded at the sequence edges).
Weights are replicated (streamed from HBM once per core, bf16).

Band-softmax identity used (reference softmaxes the FULL row with zeros
outside the band):
    out_i = (sum_band (e^{s_ij} - 1) V_j + sum_all V_j) / (sum_band (e^{s_ij} - 1) + N)
computed per head with V_raw = x@Wv (no bias; bv is folded in after the
attention average), bk applied only to real (non-padding) K rows via an
indicator-row matmul, and the global sum_all V_j = (sum_n x_n) @ Wv computed
on-device from the (tiny) host-reduced x column-sum.

Compute dtype: bf16 operands into the PE (fp32 runs at quarter rate on TRN2 —
two half-speed passes), fp32 PSUM accumulation and fp32 softmax arithmetic.
"""

import numpy as np
import ml_dtypes
from contextlib import ExitStack

import concourse.bass as bass
import concourse.tile as tile
from concourse import bacc, mybir
from concourse.bass_utils import run_bass_kernel_spmd
from concourse.masks import make_identity

F32 = mybir.dt.float32
BF16 = mybir.dt.bfloat16
NPBF16 = ml_dtypes.bfloat16
N, E, H, D = 2048, 1024, 16, 64
R = N // 8          # 256 query rows per core
HALO = R + 128      # 384 K/V rows per core
NQB = R // 128      # query blocks per core


def build_graph():
    nc = bacc.Bacc("TRN2", target_bir_lowering=False, debug=False, num_devices=8)

    xh_d = nc.declare_dram_parameter("xh", [HALO, E], BF16, isOutput=False)
    xvalid_d = nc.declare_dram_parameter("xvalid", [1, HALO], BF16, isOutput=False)
    wq_d = nc.declare_dram_parameter("Wq", [E, H * D], BF16, isOutput=False)
    wk_d = nc.declare_dram_parameter("Wk", [E, H * D], BF16, isOutput=False)
    wv_d = nc.declare_dram_parameter("Wv", [E, H * D], BF16, isOutput=False)
    wo_d = nc.declare_dram_parameter("Wo", [H * D, E], BF16, isOutput=False)
    bq_d = nc.declare_dram_parameter("bq_r", [128, 8], F32, isOutput=False)
    bk_d = nc.declare_dram_parameter("bk_row", [1, H * D], BF16, isOutput=False)
    bv_d = nc.declare_dram_parameter("bv_r", [128, 8], F32, isOutput=False)
    bo_d = nc.declare_dram_parameter("bo_row", [1, E], BF16, isOutput=False)
    xsum_d = nc.declare_dram_parameter("xsum_r", [128, 8], BF16, isOutput=False)
    m0_d = nc.declare_dram_parameter("mask0", [128, 128], F32, isOutput=False)
    m1_d = nc.declare_dram_parameter("mask1", [128, 128], F32, isOutput=False)
    out_d = nc.declare_dram_parameter("out", [R, E], F32, isOutput=True)

    with tile.TileContext(nc) as tc, ExitStack() as ctx:
        const = ctx.enter_context(tc.tile_pool(name="const", bufs=1))
        pers = ctx.enter_context(tc.tile_pool(name="pers", bufs=1))
        wpool = ctx.enter_context(tc.tile_pool(name="wpool", bufs=4))
        epool = ctx.enter_context(tc.tile_pool(name="epool", bufs=6))
        ppool = ctx.enter_context(tc.tile_pool(name="ppool", bufs=6))
        zpool = ctx.enter_context(tc.tile_pool(name="zpool", bufs=4))
        obpool = ctx.enter_context(tc.tile_pool(name="obpool", bufs=2))
        psum = ctx.enter_context(tc.tile_pool(name="psum", bufs=8, space="PSUM"))

        def ps(shape, dt=F32):
            return psum.tile(shape, dt, tag="ps", name="pst")

        # ---- constants & small inputs -------------------------------------
        identity = const.tile([128, 128], BF16, tag="identity")
        make_identity(nc, identity[:])
        m0 = const.tile([128, 128], F32, tag="m0")
        nc.sync.dma_start(m0[:], m0_d[:, :])
        m1 = const.tile([128, 128], F32, tag="m1")
        nc.sync.dma_start(m1[:], m1_d[:, :])
        bq_sb = const.tile([128, 8], F32, tag="bq")
        nc.sync.dma_start(bq_sb[:], bq_d[:, :])
        bv_sb = const.tile([128, 8], F32, tag="bv")
        nc.sync.dma_start(bv_sb[:], bv_d[:, :])
        xsum_sb = const.tile([128, 8], BF16, tag="xsum")
        nc.sync.dma_start(xsum_sb[:], xsum_d[:, :])
        bk_sb = const.tile([1, H * D], BF16, tag="bk")
        nc.sync.dma_start(bk_sb[:], bk_d[:, :])
        bo_sb = const.tile([1, E], BF16, tag="bo")
        nc.sync.dma_start(bo_sb[:], bo_d[:, :])
        valid_sb = const.tile([1, HALO], BF16, tag="valid")
        nc.sync.dma_start(valid_sb[:], xvalid_d[:, :])
        ones_sb = const.tile([1, 128], BF16, tag="ones")
        nc.vector.memset(ones_sb[:], 1.0)
        biascat = const.tile([1, H, D + 1], BF16, tag="biascat")

        # ---- persistent activations ---------------------------------------
        xT = pers.tile([128, 8, HALO], BF16, tag="xT")       # [e_p, e_t, seq]
        QT = pers.tile([128, 8, R], BF16, tag="QT")          # [d_p, d_t, q]
        KT = pers.tile([128, 8, HALO], BF16, tag="KT")       # [d_p, d_t, seq]
        Vaug = pers.tile([128, 3, H, D + 1], BF16, tag="Vaug")
        Asc = pers.tile([128, NQB, H * D], BF16, tag="Asc")  # [q_p, qblk, dims]
        AT = pers.tile([128, 8, R], BF16, tag="AT")          # [d_p, d_t, q]

        # ---- load x and transpose to xT (PE transpose path) ---------------
        USE_DMA_TRANSPOSE = False
        if USE_DMA_TRANSPOSE:
            for et in range(8):
                nc.sync.dma_start(xT[:, et, :], xh_d[:, et * 128:(et + 1) * 128],
                                  transpose=True)
        else:
            xtiles = []
            for st in range(3):
                xt = wpool.tile([128, E], BF16, tag="xload", name="xt")
                nc.sync.dma_start(xt[:], xh_d[st * 128:(st + 1) * 128, :])
                xtiles.append(xt)
            for st in range(3):
                for et in range(8):
                    tp = ps([128, 128], BF16)
                    nc.tensor.transpose(tp[:], xtiles[st][:, et * 128:(et + 1) * 128],
                                        identity[:])
                    nc.scalar.copy(xT[:, et, st * 128:(st + 1) * 128], tp[:])

        # ---- Q^T projection (dims on partitions), bias bq on copyback -----
        qps = [ps([128, R]) for _ in range(8)]
        for et in range(8):
            wt = wpool.tile([128, H * D], BF16, tag="w")
            nc.sync.dma_start(wt[:], wq_d[et * 128:(et + 1) * 128, :])
            for db in range(8):
                nc.tensor.matmul(qps[db][:], wt[:, db * 128:(db + 1) * 128],
                                 xT[:, et, 64:64 + R],
                                 start=(et == 0), stop=(et == 7))
        for db in range(8):
            nc.scalar.add(QT[:, db, :], qps[db][:], bq_sb[:, db:db + 1])

        # ---- K^T projection; bk added only on real rows via indicator mm --
        kps = [ps([128, HALO]) for _ in range(8)]
        for et in range(8):
            wt = wpool.tile([128, H * D], BF16, tag="w")
            nc.sync.dma_start(wt[:], wk_d[et * 128:(et + 1) * 128, :])
            for db in range(8):
                nc.tensor.matmul(kps[db][:], wt[:, db * 128:(db + 1) * 128],
                                 xT[:, et, :], start=(et == 0), stop=False)
        for db in range(8):
            nc.tensor.matmul(kps[db][:], bk_sb[0:1, db * 128:(db + 1) * 128],
                             valid_sb[0:1, :], start=False, stop=True)
        for db in range(8):
            nc.scalar.copy(KT[:, db, :], kps[db][:])

        # ---- V (natural layout, raw: bv folded into output stage) ---------
        # also S_V = xsum @ Wv accumulated in a [1, 1024] psum pair
        vps = [ps([128, 512]) for _ in range(6)]
        svps = [ps([1, 512]) for _ in range(2)]
        for et in range(8):
            wt = wpool.tile([128, H * D], BF16, tag="w")
            nc.sync.dma_start(wt[:], wv_d[et * 128:(et + 1) * 128, :])
            for st in range(3):
                for hf in range(2):
                    nc.tensor.matmul(vps[st * 2 + hf][:],
                                     xT[:, et, st * 128:(st + 1) * 128],
                                     wt[:, hf * 512:(hf + 1) * 512],
                                     start=(et == 0), stop=(et == 7))
            for hf in range(2):
                nc.tensor.matmul(svps[hf][:], xsum_sb[:, et:et + 1],
                                 wt[:, hf * 512:(hf + 1) * 512],
                                 start=(et == 0), stop=(et == 7))
        for st in range(3):
            for hf in range(2):
                src = vps[st * 2 + hf][:].rearrange("p (h d) -> p h d", d=D)
                nc.scalar.copy(Vaug[:, st, hf * 8:(hf + 1) * 8, 0:D], src)
        nc.vector.memset(Vaug[:, :, :, D:D + 1], 1.0)
        for hf in range(2):
            src = svps[hf][:].rearrange("p (h d) -> p h d", d=D)
            nc.scalar.copy(biascat[:, hf * 8:(hf + 1) * 8, 0:D], src)
        nc.vector.memset(biascat[:, :, D:D + 1], 2048.0)

        # ---- banded attention (software-pipelined, lag PV behind S) -------
        # Per head: S^T = K^T.T @ Q^T (2 matmuls) -> exp on ACT -> (-1) on
        # GpSimd -> band-mask multiply on DVE -> PV matmuls. Emitting the PV
        # matmuls LAG heads behind the S matmuls keeps the PE streaming
        # instead of stalling on the ACT/DVE chain between S and PV.
        LAG = 2
        for qblk in range(NQB):
            ptiles = {}
            pvts = {}

            def smm_stage(h):
                db, r = h // 2, (h % 2) * 64
                ptiles[h] = []
                for cblk in range(2):
                    sp = ps([128, 128])
                    nc.tensor.matmul(
                        sp[:],
                        KT[r:r + 64, db, (qblk + cblk) * 128:(qblk + cblk + 1) * 128],
                        QT[r:r + 64, db, qblk * 128:(qblk + 1) * 128],
                        start=True, stop=True)
                    et_ = epool.tile([128, 128], F32, tag="e", name="et_")
                    nc.scalar.activation(et_[:], sp[:],
                                         mybir.ActivationFunctionType.Exp)
                    nc.vector.tensor_scalar_add(et_[:], et_[:], -1.0)
                    pt = ppool.tile([128, 128], BF16, tag="p", name="pt")
                    nc.vector.tensor_mul(pt[:], et_[:],
                                         (m0 if cblk == 0 else m1)[:])
                    ptiles[h].append(pt)

            def pv_stage(h):
                pair = h // 2
                if h % 2 == 0:
                    pvts[pair] = ps([128, 2 * (D + 1)])
                pv = pvts[pair]
                off = (h % 2) * (D + 1)
                for cblk in range(2):
                    # one accumulation group per PSUM bank (two heads share
                    # a bank): only the bank's first matmul starts it
                    nc.tensor.matmul(pv[:, off:off + D + 1],
                                     ptiles[h][cblk][:],
                                     Vaug[:, qblk + cblk, h, :],
                                     start=(h % 2 == 0 and cblk == 0),
                                     stop=False)
                if h % 2 == 1:
                    # [S_V | 2048] added to every query row (rank-1 ones mm)
                    for hh in (h - 1, h):
                        offh = (hh % 2) * (D + 1)
                        nc.tensor.matmul(pv[:, offh:offh + D + 1],
                                         ones_sb[0:1, :], biascat[0:1, hh, :],
                                         start=False, stop=(hh % 2 == 1))
                    for hh in (h - 1, h):
                        offh = (hh % 2) * (D + 1)
                        zinv = zpool.tile([128, 1], F32, tag="z", name="zinv")
                        nc.vector.reciprocal(zinv[:], pv[:, offh + D:offh + D + 1])
                        nc.vector.tensor_scalar_mul(
                            Asc[:, qblk, hh * D:(hh + 1) * D],
                            pv[:, offh:offh + D], zinv[:])
                    del ptiles[h - 1], ptiles[h], pvts[pair]

            for step in range(H + LAG):
                if step < H:
                    smm_stage(step)
                if step >= LAG:
                    pv_stage(step - LAG)

        # ---- transpose A (and add bv) for the output projection -----------
        for qblk in range(NQB):
            for at in range(8):
                tp = ps([128, 128], BF16)
                nc.tensor.transpose(tp[:], Asc[:, qblk, at * 128:(at + 1) * 128],
                                    identity[:])
                nc.scalar.add(AT[:, at, qblk * 128:(qblk + 1) * 128], tp[:],
                              bv_sb[:, at:at + 1])

        # ---- output projection: O = (A + bv) @ Wo + bo --------------------
        ops = [ps([128, 512]) for _ in range(2 * NQB)]
        for qblk in range(NQB):
            for hf in range(2):
                nc.tensor.matmul(ops[qblk * 2 + hf][:], ones_sb[0:1, :],
                                 bo_sb[0:1, hf * 512:(hf + 1) * 512],
                                 start=True, stop=False)
        for at in range(8):
            wt = wpool.tile([128, E], BF16, tag="w")
            nc.sync.dma_start(wt[:], wo_d[at * 128:(at + 1) * 128, :])
            for qblk in range(NQB):
                for hf in range(2):
                    nc.tensor.matmul(ops[qblk * 2 + hf][:],
                                     AT[:, at, qblk * 128:(qblk + 1) * 128],
                                     wt[:, hf * 512:(hf + 1) * 512],
                                     start=False, stop=(at == 7))
        for qblk in range(NQB):
            ob = obpool.tile([128, E], F32, tag="ob")
            for hf in range(2):
                nc.scalar.copy(ob[:, hf * 512:(hf + 1) * 512],
                               ops[qblk * 2 + hf][:])
            nc.sync.dma_start(out_d[qblk * 128:(qblk + 1) * 128, :], ob[:])

    nc.compile()
    return nc


_NC = None


def get_nc():
    global _NC
    if _NC is None:
        _NC = build_graph()
    return _NC


def make_in_maps(x, Wq, bq, Wk, bk, Wv, bv, Wo, bo):
    f = lambda a: np.ascontiguousarray(np.asarray(a, dtype=np.float32))
    bf = lambda a: np.ascontiguousarray(
        np.asarray(a, dtype=np.float32).astype(NPBF16))
    x2 = f(x).reshape(N, E)
    ci = np.arange(128, dtype=np.float32)[:, None]
    qi = np.arange(128, dtype=np.float32)[None, :]
    common = {
        "Wq": bf(Wq), "Wk": bf(Wk), "Wv": bf(Wv), "Wo": bf(Wo),
        "bq_r": f(bq).reshape(8, 128).T.copy(),
        "bk_row": bf(bk).reshape(1, H * D),
        "bv_r": f(bv).reshape(8, 128).T.copy(),
        "bo_row": bf(bo).reshape(1, E),
        "xsum_r": bf(x2.sum(0, dtype=np.float32)).reshape(8, 128).T.copy(),
        "mask0": (ci >= qi).astype(np.float32),
        "mask1": (ci <= qi).astype(np.float32),
    }
    in_maps = []
    for c in range(8):
        r0 = c * R
        xh = np.zeros((HALO, E), NPBF16)
        valid = np.zeros((1, HALO), NPBF16)
        lo, hi = r0 - 64, r0 + R + 64
        slo, shi = max(lo, 0), min(hi, N)
        xh[slo - lo: shi - lo] = x2[slo:shi].astype(NPBF16)
        valid[0, slo - lo: shi - lo] = 1.0
        in_maps.append({**common, "xh": xh, "xvalid": valid})
    return in_maps


def kernel(x, Wq, bq, Wk, bk, Wv, bv, Wo, bo, _trace=False, _trace_kwargs=None):
    nc = get_nc()
    in_maps = make_in_maps(x, Wq, bq, Wk, bk, Wv, bv, Wo, bo)
    res = run_bass_kernel_spmd(nc, in_maps, list(range(8)), trace=_trace,
                               **(_trace_kwargs or {}))
    out = np.concatenate([res.results[c]["out"] for c in range(8)], axis=0)
    kernel.last_result = res
    return out[None].astype(np.float32)


# revision 10
# speedup vs baseline: 1.6849x; 1.0788x over previous
"""Multi-head dilated sliding-window attention (window=129, dil=1) on 8 TRN2 cores.

Sharding: sequence-parallel. Each core computes 256 query rows (N=2048 / 8),
with a 64-row K/V halo on each side (zero-padded at the sequence edges).
Weights are replicated (resident in SBUF, bf16).

Band-softmax identity used (reference softmaxes the FULL row with zeros
outside the band):
    out_i = (sum_band (e^{s_ij} - 1) V_j + sum_all V_j) / (sum_band (e^{s_ij} - 1) + N)
computed per head with V_raw = x@Wv (no bias; bv is folded in after the
attention average), bk applied only to real (non-padding) K rows via an
indicator-row matmul, and the global sum_all V_j = (sum_n x_n) @ Wv computed
on-device from the (tiny) host-reduced x column-sum.

Compute dtype: bf16 operands into the PE (fp32 runs at quarter rate on TRN2),
fp32 PSUM accumulation and fp32 softmax arithmetic.

Structure: Q^T/K^T projections are computed per head-pair (db) and attention
for that pair runs immediately, pipelined one round behind the scores so the
PE never stalls on the ACT/DVE softmax chain.
"""

import numpy as np
import ml_dtypes
from contextlib import ExitStack

import concourse.bass as bass
import concourse.tile as tile
from concourse import bacc, mybir
from concourse.bass_utils import run_bass_kernel_spmd

F32 = mybir.dt.float32
BF16 = mybir.dt.bfloat16
NPBF16 = ml_dtypes.bfloat16
N, E, H, D = 2048, 1024, 16, 64
R = N // 8          # 256 query rows per core
HALO = R + 128      # 384 K/V rows per core
NQB = R // 128      # query blocks per core


def build_graph():
    nc = bacc.Bacc("TRN2", target_bir_lowering=False, debug=False, num_devices=8)

    xh_d = nc.declare_dram_parameter("xh", [HALO, E], BF16, isOutput=False)
    xvalid_d = nc.declare_dram_parameter("xvalid", [1, HALO], BF16, isOutput=False)
    wq_d = nc.declare_dram_parameter("Wq", [E, H * D], BF16, isOutput=False)
    wk_d = nc.declare_dram_parameter("Wk", [E, H * D], BF16, isOutput=False)
    wv_d = nc.declare_dram_parameter("Wv", [E, H * D], BF16, isOutput=False)
    wo_d = nc.declare_dram_parameter("Wo", [H * D, E], BF16, isOutput=False)
    bq_d = nc.declare_dram_parameter("bq_r", [128, 8], F32, isOutput=False)
    bk_d = nc.declare_dram_parameter("bk_row", [1, H * D], BF16, isOutput=False)
    bv_d = nc.declare_dram_parameter("bv_r", [128, 8], F32, isOutput=False)
    bo_d = nc.declare_dram_parameter("bo_row", [1, E], BF16, isOutput=False)
    xsum_d = nc.declare_dram_parameter("xsum_r", [128, 8], BF16, isOutput=False)
    m2_d = nc.declare_dram_parameter("mask2", [128, 256], F32, isOutput=False)
    id_d = nc.declare_dram_parameter("ident", [128, 128], BF16, isOutput=False)
    out_d = nc.declare_dram_parameter("out", [R, E], F32, isOutput=True)

    with tile.TileContext(nc) as tc, ExitStack() as ctx:
        const = ctx.enter_context(tc.tile_pool(name="const", bufs=1))
        pers = ctx.enter_context(tc.tile_pool(name="pers", bufs=1))
        epool = ctx.enter_context(tc.tile_pool(name="epool", bufs=4))
        ppool = ctx.enter_context(tc.tile_pool(name="ppool", bufs=6))
        zpool = ctx.enter_context(tc.tile_pool(name="zpool", bufs=4))
        obpool = ctx.enter_context(tc.tile_pool(name="obpool", bufs=2))
        psum = ctx.enter_context(tc.tile_pool(name="psum", bufs=8, space="PSUM"))

        def ps(shape, dt=F32):
            return psum.tile(shape, dt, tag="ps", name="pst")

        # ---- loads: x first (critical path), then resident weights --------
        xtiles = []
        for st in range(3):
            xt = const.tile([128, E], BF16, tag=f"xload{st}", name="xt")
            nc.sync.dma_start(xt[:], xh_d[st * 128:(st + 1) * 128, :])
            xtiles.append(xt)
        identity = const.tile([128, 128], BF16, tag="identity")
        nc.sync.dma_start(identity[:], id_d[:, :])

        wq_t, wk_t, wv_t, wo_t = [], [], [], []
        for et in range(8):
            for lst, src, nm in ((wq_t, wq_d, "wq"), (wk_t, wk_d, "wk"),
                                 (wv_t, wv_d, "wv"), (wo_t, wo_d, "wo")):
                wt = const.tile([128, E], BF16, tag=f"{nm}{et}", name="wt")
                nc.sync.dma_start(wt[:], src[et * 128:(et + 1) * 128, :])
                lst.append(wt)

        m2 = const.tile([128, 256], F32, tag="m2")
        nc.sync.dma_start(m2[:], m2_d[:, :])
        bq_sb = const.tile([128, 8], F32, tag="bq")
        nc.sync.dma_start(bq_sb[:], bq_d[:, :])
        bv_sb = const.tile([128, 8], F32, tag="bv")
        nc.sync.dma_start(bv_sb[:], bv_d[:, :])
        xsum_sb = const.tile([128, 8], BF16, tag="xsum")
        nc.sync.dma_start(xsum_sb[:], xsum_d[:, :])
        bk_sb = const.tile([1, H * D], BF16, tag="bk")
        nc.sync.dma_start(bk_sb[:], bk_d[:, :])
        bo_sb = const.tile([1, E], BF16, tag="bo")
        nc.sync.dma_start(bo_sb[:], bo_d[:, :])
        valid_sb = const.tile([1, HALO], BF16, tag="valid")
        nc.sync.dma_start(valid_sb[:], xvalid_d[:, :])
        ones_sb = const.tile([1, 128], BF16, tag="ones")
        nc.vector.memset(ones_sb[:], 1.0)
        biascat = const.tile([1, H, D + 1], BF16, tag="biascat")

        # ---- persistent activations ---------------------------------------
        xT = pers.tile([128, 8, HALO], BF16, tag="xT")       # [e_p, e_t, seq]
        QT = pers.tile([128, 8, R], BF16, tag="QT")          # [d_p, d_t, q]
        KT = pers.tile([128, 8, HALO], BF16, tag="KT")       # [d_p, d_t, seq]
        Vaug = pers.tile([128, 3, H, D + 1], BF16, tag="Vaug")
        Asc = pers.tile([128, NQB, H * D], BF16, tag="Asc")  # [q_p, qblk, dims]
        AT = pers.tile([128, 8, R], BF16, tag="AT")          # [d_p, d_t, q]

        # ---- transpose x to xT (PE transpose) -----------------------------
        for st in range(3):
            for et in range(8):
                tp = ps([128, 128], BF16)
                nc.tensor.transpose(tp[:], xtiles[st][:, et * 128:(et + 1) * 128],
                                    identity[:])
                nc.scalar.copy(xT[:, et, st * 128:(st + 1) * 128], tp[:])

        # ---- V (natural layout, raw) + S_V = xsum @ Wv --------------------
        vps = [ps([128, 512]) for _ in range(6)]
        svps = [ps([1, 512]) for _ in range(2)]
        for et in range(8):
            for st in range(3):
                for hf in range(2):
                    nc.tensor.matmul(vps[st * 2 + hf][:],
                                     xT[:, et, st * 128:(st + 1) * 128],
                                     wv_t[et][:, hf * 512:(hf + 1) * 512],
                                     start=(et == 0), stop=(et == 7))
            for hf in range(2):
                nc.tensor.matmul(svps[hf][:], xsum_sb[:, et:et + 1],
                                 wv_t[et][:, hf * 512:(hf + 1) * 512],
                                 start=(et == 0), stop=(et == 7))
        for st in range(3):
            for hf in range(2):
                src = vps[st * 2 + hf][:].rearrange("p (h d) -> p h d", d=D)
                nc.scalar.copy(Vaug[:, st, hf * 8:(hf + 1) * 8, 0:D], src)
        nc.vector.memset(Vaug[:, :, :, D:D + 1], 1.0)
        for hf in range(2):
            src = svps[hf][:].rearrange("p (h d) -> p h d", d=D)
            nc.scalar.copy(biascat[:, hf * 8:(hf + 1) * 8, 0:D], src)
        nc.vector.memset(biascat[:, :, D:D + 1], 2048.0)

        # ---- fused projections + banded attention, one head-pair at a time
        # round r = (db, qblk). Emission order inside round r:
        #   1. Q^T/K^T projection matmuls for db (only when qblk == 0)
        #   2. PV + bias matmuls and epilogue of round r-1 (p tiles ready)
        #   3. S matmuls + exp/-1/mask chain for round r
        prev = None  # (db, qblk, ptiles{h: pt}, pv psum)

        def proj(db):
            qp = ps([128, R])
            for et in range(8):
                nc.tensor.matmul(qp[:], wq_t[et][:, db * 128:(db + 1) * 128],
                                 xT[:, et, 64:64 + R],
                                 start=(et == 0), stop=(et == 7))
            nc.scalar.add(QT[:, db, :], qp[:], bq_sb[:, db:db + 1])
            kp = ps([128, HALO])
            for et in range(8):
                nc.tensor.matmul(kp[:], wk_t[et][:, db * 128:(db + 1) * 128],
                                 xT[:, et, :], start=(et == 0), stop=False)
            nc.tensor.matmul(kp[:], bk_sb[0:1, db * 128:(db + 1) * 128],
                             valid_sb[0:1, :], start=False, stop=True)
            nc.scalar.copy(KT[:, db, :], kp[:])

        def pv_flush(pr):
            db, qblk, ptl, pv = pr
            for i, h in enumerate((2 * db, 2 * db + 1)):
                off = i * (D + 1)
                for cblk in range(2):
                    nc.tensor.matmul(pv[:, off:off + D + 1],
                                     ptl[h][:, cblk * 128:(cblk + 1) * 128],
                                     Vaug[:, qblk + cblk, h, :],
                                     start=(i == 0 and cblk == 0), stop=False)
            for i, h in enumerate((2 * db, 2 * db + 1)):
                off = i * (D + 1)
                nc.tensor.matmul(pv[:, off:off + D + 1], ones_sb[0:1, :],
                                 biascat[0:1, h, :], start=False, stop=(i == 1))
            for i, h in enumerate((2 * db, 2 * db + 1)):
                off = i * (D + 1)
                zinv = zpool.tile([128, 1], F32, tag="z", name="zinv")
                nc.vector.reciprocal(zinv[:], pv[:, off + D:off + D + 1])
                nc.scalar.activation(Asc[:, qblk, h * D:(h + 1) * D],
                                     pv[:, off:off + D],
                                     mybir.ActivationFunctionType.Copy,
                                     scale=zinv[:])

        for r in range(2 * 8 + 1):
            db, qblk = r // 2, r % 2
            if r < 16 and qblk == 0:
                proj(db)
            if r < 16:
                ptl = {}
                pv = ps([128, 2 * (D + 1)])
                if prev is not None:
                    pv_flush(prev)
                for i, h in enumerate((2 * db, 2 * db + 1)):
                    rr = i * 64
                    sp = ps([128, 256])
                    for cblk in range(2):
                        nc.tensor.matmul(
                            sp[:, cblk * 128:(cblk + 1) * 128],
                            KT[rr:rr + 64, db,
                               (qblk + cblk) * 128:(qblk + cblk + 1) * 128],
                            QT[rr:rr + 64, db, qblk * 128:(qblk + 1) * 128],
                            start=(cblk == 0), stop=(cblk == 1))
                    et_ = epool.tile([128, 256], F32, tag="e", name="et_")
                    nc.scalar.activation(et_[:], sp[:],
                                         mybir.ActivationFunctionType.Exp)
                    nc.vector.tensor_scalar_add(et_[:], et_[:], -1.0)
                    pt = ppool.tile([128, 256], BF16, tag="p", name="pt")
                    nc.vector.tensor_mul(pt[:], et_[:], m2[:])
                    ptl[h] = pt
                prev = (db, qblk, ptl, pv)
            else:
                pv_flush(prev)

        # ---- transpose A (and add bv) for the output projection -----------
        for qblk in range(NQB):
            for at in range(8):
                tp = ps([128, 128], BF16)
                nc.tensor.transpose(tp[:], Asc[:, qblk, at * 128:(at + 1) * 128],
                                    identity[:])
                nc.scalar.add(AT[:, at, qblk * 128:(qblk + 1) * 128], tp[:],
                              bv_sb[:, at:at + 1])

        # ---- output projection: O = (A + bv) @ Wo + bo --------------------
        ops = [ps([128, 512]) for _ in range(2 * NQB)]
        for qblk in range(NQB):
            for hf in range(2):
                nc.tensor.matmul(ops[qblk * 2 + hf][:], ones_sb[0:1, :],
                                 bo_sb[0:1, hf * 512:(hf + 1) * 512],
                                 start=True, stop=False)
        for at in range(8):
            for qblk in range(NQB):
                for hf in range(2):
                    nc.tensor.matmul(ops[qblk * 2 + hf][:],
                                     AT[:, at, qblk * 128:(qblk + 1) * 128],
                                     wo_t[at][:, hf * 512:(hf + 1) * 512],
                                     start=False, stop=(at == 7))
        for qblk in range(NQB):
            ob = obpool.tile([128, E], F32, tag="ob")
            for hf in range(2):
                nc.scalar.copy(ob[:, hf * 512:(hf + 1) * 512],
                               ops[qblk * 2 + hf][:])
            nc.sync.dma_start(out_d[qblk * 128:(qblk + 1) * 128, :], ob[:])

    nc.compile()
    return nc


_NC = None


def get_nc():
    global _NC
    if _NC is None:
        _NC = build_graph()
    return _NC


def make_in_maps(x, Wq, bq, Wk, bk, Wv, bv, Wo, bo):
    f = lambda a: np.ascontiguousarray(np.asarray(a, dtype=np.float32))
    bf = lambda a: np.ascontiguousarray(
        np.asarray(a, dtype=np.float32).astype(NPBF16))
    x2 = f(x).reshape(N, E)
    ci = np.arange(128, dtype=np.float32)[:, None]  # key index c (partitions)
    qi = np.arange(128, dtype=np.float32)[None, :]  # query index q (free)
    mask2 = np.concatenate([(ci >= qi).astype(np.float32),
                            (ci <= qi).astype(np.float32)], axis=1)
    common = {
        "Wq": bf(Wq), "Wk": bf(Wk), "Wv": bf(Wv), "Wo": bf(Wo),
        "bq_r": f(bq).reshape(8, 128).T.copy(),
        "bk_row": bf(bk).reshape(1, H * D),
        "bv_r": f(bv).reshape(8, 128).T.copy(),
        "bo_row": bf(bo).reshape(1, E),
        "xsum_r": bf(x2.sum(0, dtype=np.float32)).reshape(8, 128).T.copy(),
        "mask2": np.ascontiguousarray(mask2),
        "ident": np.eye(128, dtype=np.float32).astype(NPBF16),
    }
    in_maps = []
    for c in range(8):
        r0 = c * R
        xh = np.zeros((HALO, E), NPBF16)
        valid = np.zeros((1, HALO), NPBF16)
        lo, hi = r0 - 64, r0 + R + 64
        slo, shi = max(lo, 0), min(hi, N)
        xh[slo - lo: shi - lo] = x2[slo:shi].astype(NPBF16)
        valid[0, slo - lo: shi - lo] = 1.0
        in_maps.append({**common, "xh": xh, "xvalid": valid})
    return in_maps


def kernel(x, Wq, bq, Wk, bk, Wv, bv, Wo, bo, _trace=False, _trace_kwargs=None):
    nc = get_nc()
    in_maps = make_in_maps(x, Wq, bq, Wk, bk, Wv, bv, Wo, bo)
    res = run_bass_kernel_spmd(nc, in_maps, list(range(8)), trace=_trace,
                               **(_trace_kwargs or {}))
    out = np.concatenate([res.results[c]["out"] for c in range(8)], axis=0)
    kernel.last_result = res
    return out[None].astype(np.float32)


# revision 13
# speedup vs baseline: 2.1374x; 1.2686x over previous
"""Multi-head dilated sliding-window attention (window=129, dil=1) on 8 TRN2 cores.

Sharding: sequence-parallel. Each core computes 256 query rows (N=2048 / 8),
with a 64-row K/V halo on each side (zero-padded at the sequence edges).
Weights are replicated (resident in SBUF, bf16).

Band-softmax identity used (reference softmaxes the FULL row with zeros
outside the band):
    out_i = (sum_band (e^{s_ij} - 1) V_j + sum_all V_j) / (sum_band (e^{s_ij} - 1) + N)
computed per head with V_raw = x@Wv (no bias; bv is folded in after the
attention average), bk applied only to real (non-padding) K rows via an
indicator-row matmul, and the global sum_all V_j = (sum_n x_n) @ Wv computed
on-device from the (tiny) host-reduced x column-sum.

Compute dtype: bf16 operands into the PE (fp32 runs at quarter rate on TRN2),
fp32 PSUM accumulation and fp32 softmax arithmetic.

Structure: Q^T/K^T projections are computed per head-pair (db) and attention
for that pair runs immediately, pipelined one round behind the scores so the
PE never stalls on the ACT/DVE softmax chain.
"""

import numpy as np
import ml_dtypes
from contextlib import ExitStack

import concourse.bass as bass
import concourse.tile as tile
from concourse import bacc, mybir
from concourse.bass_utils import run_bass_kernel_spmd

F32 = mybir.dt.float32
BF16 = mybir.dt.bfloat16
NPBF16 = ml_dtypes.bfloat16
N, E, H, D = 2048, 1024, 16, 64
R = N // 8          # 256 query rows per core
HALO = R + 128      # 384 K/V rows per core
NQB = R // 128      # query blocks per core


def build_graph():
    nc = bacc.Bacc("TRN2", target_bir_lowering=False, debug=False, num_devices=8)

    xh_d = nc.declare_dram_parameter("xh", [HALO, E], BF16, isOutput=False)
    xvalid_d = nc.declare_dram_parameter("xvalid", [1, HALO], BF16, isOutput=False)
    wq_d = nc.declare_dram_parameter("Wq", [E, H * D], BF16, isOutput=False)
    wk_d = nc.declare_dram_parameter("Wk", [E, H * D], BF16, isOutput=False)
    wv_d = nc.declare_dram_parameter("Wv", [E, H * D], BF16, isOutput=False)
    wo_d = nc.declare_dram_parameter("Wo", [H * D, E], BF16, isOutput=False)
    bq_d = nc.declare_dram_parameter("bq_r", [128, 8], F32, isOutput=False)
    bk_d = nc.declare_dram_parameter("bk_row", [1, H * D], BF16, isOutput=False)
    bv_d = nc.declare_dram_parameter("bv_r", [128, 8], F32, isOutput=False)
    bo_d = nc.declare_dram_parameter("bo_row", [1, E], BF16, isOutput=False)
    xsum_d = nc.declare_dram_parameter("xsum_r", [128, 8], BF16, isOutput=False)
    m4_d = nc.declare_dram_parameter("mask4", [128, 512], F32, isOutput=False)
    id_d = nc.declare_dram_parameter("ident", [128, 128], BF16, isOutput=False)
    out_d = nc.declare_dram_parameter("out", [R, E], F32, isOutput=True)

    with tile.TileContext(nc) as tc, ExitStack() as ctx:
        const = ctx.enter_context(tc.tile_pool(name="const", bufs=1))
        pers = ctx.enter_context(tc.tile_pool(name="pers", bufs=1))
        epool = ctx.enter_context(tc.tile_pool(name="epool", bufs=3))
        ppool = ctx.enter_context(tc.tile_pool(name="ppool", bufs=5))
        zpool = ctx.enter_context(tc.tile_pool(name="zpool", bufs=4))
        obpool = ctx.enter_context(tc.tile_pool(name="obpool", bufs=2))
        psum = ctx.enter_context(tc.tile_pool(name="psum", bufs=8, space="PSUM"))

        def ps(shape, dt=F32):
            return psum.tile(shape, dt, tag="ps", name="pst")

        # ---- loads: x first (critical path), then resident weights --------
        xtiles = []
        for st in range(3):
            xt = const.tile([128, E], BF16, tag=f"xload{st}", name="xt")
            nc.sync.dma_start(xt[:], xh_d[st * 128:(st + 1) * 128, :])
            xtiles.append(xt)
        identity = const.tile([128, 128], BF16, tag="identity")
        nc.sync.dma_start(identity[:], id_d[:, :])

        # ---- PE clock warm-up: ~3.5us of dummy matmuls during the DMA
        # phase so the HAM clock gate is already at 8/8 when real work lands
        wu = const.tile([128, 128], BF16, tag="wu")
        nc.vector.memset(wu[:], 0.0)
        wups = psum.tile([128, 128], F32, tag="ps", name="wups")
        for _ in range(32):
            nc.tensor.matmul(wups[:], wu[:], wu[:], start=True, stop=True)

        wq_t, wk_t, wv_t, wo_t = [], [], [], []
        for et in range(8):
            for lst, src, nm in ((wq_t, wq_d, "wq"), (wk_t, wk_d, "wk"),
                                 (wv_t, wv_d, "wv"), (wo_t, wo_d, "wo")):
                wt = const.tile([128, E], BF16, tag=f"{nm}{et}", name="wt")
                nc.sync.dma_start(wt[:], src[et * 128:(et + 1) * 128, :])
                lst.append(wt)

        m4 = const.tile([128, 512], F32, tag="m4")
        nc.sync.dma_start(m4[:], m4_d[:, :])
        bq_sb = const.tile([128, 8], F32, tag="bq")
        nc.sync.dma_start(bq_sb[:], bq_d[:, :])
        bv_sb = const.tile([128, 8], F32, tag="bv")
        nc.sync.dma_start(bv_sb[:], bv_d[:, :])
        xsum_sb = const.tile([128, 8], BF16, tag="xsum")
        nc.sync.dma_start(xsum_sb[:], xsum_d[:, :])
        bk_sb = const.tile([1, H * D], BF16, tag="bk")
        nc.sync.dma_start(bk_sb[:], bk_d[:, :])
        bo_sb = const.tile([1, E], BF16, tag="bo")
        nc.sync.dma_start(bo_sb[:], bo_d[:, :])
        valid_sb = const.tile([1, HALO], BF16, tag="valid")
        nc.sync.dma_start(valid_sb[:], xvalid_d[:, :])
        ones_sb = const.tile([1, 128], BF16, tag="ones")
        nc.vector.memset(ones_sb[:], 1.0)
        biascat = const.tile([1, H, D + 1], BF16, tag="biascat")

        # ---- persistent activations ---------------------------------------
        xT = pers.tile([128, 8, HALO], BF16, tag="xT")       # [e_p, e_t, seq]
        QT = pers.tile([128, 8, R], BF16, tag="QT")          # [d_p, d_t, q]
        KT = pers.tile([128, 8, HALO], BF16, tag="KT")       # [d_p, d_t, seq]
        Vaug = pers.tile([128, 3, H, D + 1], BF16, tag="Vaug")
        Asc = pers.tile([128, NQB, H * D], BF16, tag="Asc")  # [q_p, qblk, dims]
        AT = pers.tile([128, 8, R], BF16, tag="AT")          # [d_p, d_t, q]

        # ---- transpose x to xT (PE transpose) -----------------------------
        for st in range(3):
            for et in range(8):
                tp = ps([128, 128], BF16)
                nc.tensor.transpose(tp[:], xtiles[st][:, et * 128:(et + 1) * 128],
                                    identity[:])
                nc.vector.tensor_copy(xT[:, et, st * 128:(st + 1) * 128], tp[:])

        # ---- V (natural layout, raw) + S_V = xsum @ Wv --------------------
        vps = [ps([128, 512]) for _ in range(6)]
        svps = [ps([1, 512]) for _ in range(2)]
        for et in range(8):
            for st in range(3):
                for hf in range(2):
                    nc.tensor.matmul(vps[st * 2 + hf][:],
                                     xT[:, et, st * 128:(st + 1) * 128],
                                     wv_t[et][:, hf * 512:(hf + 1) * 512],
                                     start=(et == 0), stop=(et == 7))
            for hf in range(2):
                nc.tensor.matmul(svps[hf][:], xsum_sb[:, et:et + 1],
                                 wv_t[et][:, hf * 512:(hf + 1) * 512],
                                 start=(et == 0), stop=(et == 7))
        for st in range(3):
            for hf in range(2):
                src = vps[st * 2 + hf][:].rearrange("p (h d) -> p h d", d=D)
                nc.scalar.copy(Vaug[:, st, hf * 8:(hf + 1) * 8, 0:D], src)
        nc.vector.memset(Vaug[:, :, :, D:D + 1], 1.0)
        for hf in range(2):
            src = svps[hf][:].rearrange("p (h d) -> p h d", d=D)
            nc.scalar.copy(biascat[:, hf * 8:(hf + 1) * 8, 0:D], src)
        nc.vector.memset(biascat[:, :, D:D + 1], 2048.0)

        # ---- fused projections + banded attention, one head-pair at a time
        # round r = db (one head pair, BOTH query blocks). Emission order:
        #   1. Q^T/K^T projection matmuls for db
        #   2. PV + bias matmuls and epilogue of round r-1 (p tiles ready)
        #   3. S matmuls (one [128, 512] psum per head = both qblk/cblk
        #      quadrants) + exp/-1/mask chain for round r
        # Per-head p layout: [q0c0 | q0c1 | q1c0 | q1c1], quadrant j uses
        # keys halo block (qblk+cblk) and mask m0/m1 alternating.
        prev = None  # (db, ptiles{h: pt}, {qblk: pv psum})

        def proj(db):
            qp = ps([128, R])
            for et in range(8):
                nc.tensor.matmul(qp[:], wq_t[et][:, db * 128:(db + 1) * 128],
                                 xT[:, et, 64:64 + R],
                                 start=(et == 0), stop=(et == 7))
            nc.scalar.add(QT[:, db, :], qp[:], bq_sb[:, db:db + 1])
            kp = ps([128, HALO])
            for et in range(8):
                nc.tensor.matmul(kp[:], wk_t[et][:, db * 128:(db + 1) * 128],
                                 xT[:, et, :], start=(et == 0), stop=False)
            nc.tensor.matmul(kp[:], bk_sb[0:1, db * 128:(db + 1) * 128],
                             valid_sb[0:1, :], start=False, stop=True)
            nc.scalar.copy(KT[:, db, :], kp[:])

        def pv_flush(pr):
            db, ptl = pr
            pvs = {}
            for qblk in range(NQB):
                pvs[qblk] = ps([128, 2 * (D + 1)])
            for qblk in range(NQB):
                pv = pvs[qblk]
                for i, h in enumerate((2 * db, 2 * db + 1)):
                    off = i * (D + 1)
                    for cblk in range(2):
                        quad = qblk * 2 + cblk
                        nc.tensor.matmul(pv[:, off:off + D + 1],
                                         ptl[h][:, quad * 128:(quad + 1) * 128],
                                         Vaug[:, qblk + cblk, h, :],
                                         start=(i == 0 and cblk == 0),
                                         stop=False)
            for qblk in range(NQB):
                pv = pvs[qblk]
                for i, h in enumerate((2 * db, 2 * db + 1)):
                    off = i * (D + 1)
                    nc.tensor.matmul(pv[:, off:off + D + 1], ones_sb[0:1, :],
                                     biascat[0:1, h, :], start=False,
                                     stop=(i == 1))
            for qblk in range(NQB):
                pv = pvs[qblk]
                for i, h in enumerate((2 * db, 2 * db + 1)):
                    off = i * (D + 1)
                    zinv = zpool.tile([128, 1], F32, tag="z", name="zinv")
                    nc.vector.reciprocal(zinv[:], pv[:, off + D:off + D + 1])
                    nc.scalar.activation(Asc[:, qblk, h * D:(h + 1) * D],
                                         pv[:, off:off + D],
                                         mybir.ActivationFunctionType.Copy,
                                         scale=zinv[:])

        for r in range(8 + 1):
            if r < 8:
                db = r
                proj(db)
                if prev is not None:
                    pv_flush(prev)
                ptl = {}
                for i, h in enumerate((2 * db, 2 * db + 1)):
                    rr = i * 64
                    sp = ps([128, 512])
                    for quad in range(4):
                        qblk, cblk = quad // 2, quad % 2
                        nc.tensor.matmul(
                            sp[:, quad * 128:(quad + 1) * 128],
                            KT[rr:rr + 64, db,
                               (qblk + cblk) * 128:(qblk + cblk + 1) * 128],
                            QT[rr:rr + 64, db, qblk * 128:(qblk + 1) * 128],
                            start=(quad == 0), stop=(quad == 3))
                    et_ = epool.tile([128, 512], F32, tag="e", name="et_")
                    nc.scalar.activation(et_[:], sp[:],
                                         mybir.ActivationFunctionType.Exp)
                    nc.vector.tensor_scalar_add(et_[:], et_[:], -1.0)
                    pt = ppool.tile([128, 512], BF16, tag="p", name="pt")
                    nc.vector.tensor_mul(pt[:], et_[:], m4[:])
                    ptl[h] = pt
                prev = (db, ptl)
            else:
                pv_flush(prev)

        # ---- transpose A (and add bv) for the output projection -----------
        for qblk in range(NQB):
            for at in range(8):
                tp = ps([128, 128], BF16)
                nc.tensor.transpose(tp[:], Asc[:, qblk, at * 128:(at + 1) * 128],
                                    identity[:])
                nc.scalar.add(AT[:, at, qblk * 128:(qblk + 1) * 128], tp[:],
                              bv_sb[:, at:at + 1])

        # ---- output projection: O = (A + bv) @ Wo + bo --------------------
        ops = [ps([128, 512]) for _ in range(2 * NQB)]
        for qblk in range(NQB):
            for hf in range(2):
                nc.tensor.matmul(ops[qblk * 2 + hf][:], ones_sb[0:1, :],
                                 bo_sb[0:1, hf * 512:(hf + 1) * 512],
                                 start=True, stop=False)
        for at in range(8):
            for qblk in range(NQB):
                for hf in range(2):
                    nc.tensor.matmul(ops[qblk * 2 + hf][:],
                                     AT[:, at, qblk * 128:(qblk + 1) * 128],
                                     wo_t[at][:, hf * 512:(hf + 1) * 512],
                                     start=False, stop=(at == 7))
        for qblk in range(NQB):
            ob = obpool.tile([128, E], F32, tag="ob")
            for hf in range(2):
                nc.vector.tensor_copy(ob[:, hf * 512:(hf + 1) * 512],
                                      ops[qblk * 2 + hf][:])
            nc.sync.dma_start(out_d[qblk * 128:(qblk + 1) * 128, :], ob[:])

    nc.compile()
    return nc


_NC = None


def get_nc():
    global _NC
    if _NC is None:
        _NC = build_graph()
    return _NC


def make_in_maps(x, Wq, bq, Wk, bk, Wv, bv, Wo, bo):
    f = lambda a: np.ascontiguousarray(np.asarray(a, dtype=np.float32))
    bf = lambda a: np.ascontiguousarray(
        np.asarray(a, dtype=np.float32).astype(NPBF16))
    x2 = f(x).reshape(N, E)
    ci = np.arange(128, dtype=np.float32)[:, None]  # key index c (partitions)
    qi = np.arange(128, dtype=np.float32)[None, :]  # query index q (free)
    m0 = (ci >= qi).astype(np.float32)
    m1 = (ci <= qi).astype(np.float32)
    mask4 = np.concatenate([m0, m1, m0, m1], axis=1)
    common = {
        "Wq": bf(Wq), "Wk": bf(Wk), "Wv": bf(Wv), "Wo": bf(Wo),
        "bq_r": f(bq).reshape(8, 128).T.copy(),
        "bk_row": bf(bk).reshape(1, H * D),
        "bv_r": f(bv).reshape(8, 128).T.copy(),
        "bo_row": bf(bo).reshape(1, E),
        "xsum_r": bf(x2.sum(0, dtype=np.float32)).reshape(8, 128).T.copy(),
        "mask4": np.ascontiguousarray(mask4),
        "ident": np.eye(128, dtype=np.float32).astype(NPBF16),
    }
    in_maps = []
    for c in range(8):
        r0 = c * R
        xh = np.zeros((HALO, E), NPBF16)
        valid = np.zeros((1, HALO), NPBF16)
        lo, hi = r0 - 64, r0 + R + 64
        slo, shi = max(lo, 0), min(hi, N)
        xh[slo - lo: shi - lo] = x2[slo:shi].astype(NPBF16)
        valid[0, slo - lo: shi - lo] = 1.0
        in_maps.append({**common, "xh": xh, "xvalid": valid})
    return in_maps


def kernel(x, Wq, bq, Wk, bk, Wv, bv, Wo, bo, _trace=False, _trace_kwargs=None):
    nc = get_nc()
    in_maps = make_in_maps(x, Wq, bq, Wk, bk, Wv, bv, Wo, bo)
    res = run_bass_kernel_spmd(nc, in_maps, list(range(8)), trace=_trace,
                               **(_trace_kwargs or {}))
    out = np.concatenate([res.results[c]["out"] for c in range(8)], axis=0)
    kernel.last_result = res
    return out[None].astype(np.float32)
